# revision 7
# baseline (speedup 1.0000x reference)
"""CRsAE1d FISTA kernel for 8 Trainium2 NeuronCores.

Strategy
--------
H = [circ(f_0)|...|circ(f_7)] is block-circulant: with 128-row blocking each
circulant is block-bidiagonal with ONE repeated diagonal block D_k (lower-band
Toeplitz, f[0..63]) and ONE repeated subdiagonal block S_k (upper-corner band,
f[1..63]).  So H@w and H^T@v are 16 tiny [128,128] matmuls each (per filter:
one D-band + one S-band matmul whose rhs is a block-shifted view of the
operand, with circular wrap handled by an 8-column halo copy).

Data-parallel over batch: 64 columns -> 8 cores x 8 columns.  Everything lives
in SBUF; per iteration the PE does 48 fp16 matmuls (conv1 16, conv2 16,
momentum-passthrough 16 via +/- m_t scaled-identity weights accumulated into
the same PSUM region), DVE does the v-combine and softshrink
(x - clamp(x, -thr, thr)) with the f32 master copy of x kept exactly.

fp16 weights/activations with an f32 master for x gives ~4e-4 rel err vs the
f32 reference (measured in numpy bit-sim); bf16 would give ~3e-3.
"""

import sys

for p in ("/root/.axon_site", "/root/.axon_site/_ro/trn_rl_repo",
          "/root/.axon_site/_ro/pypackages", "/opt/trn_rl_repo"):
    if p not in sys.path:
        sys.path.append(p)

import numpy as np

T = 15
LAM = 0.1
N = 2048
K = 8
KS = 64
B = 64
NCORES = 8
BL = B // NCORES          # batch per core
NB = N // 128             # 16 row-blocks
CW = NB * BL              # 128 columns per (k) region in (J,b) layout
CWH = CW + BL             # body + 8-column halo

_CACHE: dict = {}


def _momentum_coeffs():
    s = 0.0
    ms = []
    for _ in range(T):
        st = (1.0 + np.sqrt(1.0 + 4.0 * s * s)) / 2.0
        ms.append(np.float32((s - 1.0) / st))
        s = st
    return ms


def _band_matrices(D):
    """D_k[r,s] = f_k[r-s] for 0<=r-s<KS;  S_k[r,s] = f_k[128+r-s] for s-r>=65."""
    Dm = np.zeros((K, 128, 128), np.float32)
    Sm = np.zeros((K, 128, 128), np.float32)
    r = np.arange(128)[:, None]
    s = np.arange(128)[None, :]
    d1 = r - s
    d2 = 128 + r - s
    m1 = (d1 >= 0) & (d1 < KS)
    m2 = (d2 > 0) & (d2 < KS)
    for k in range(K):
        Dm[k][m1] = D[k][d1[m1]]
        Sm[k][m2] = D[k][d2[m2]]
    return Dm, Sm


def _legalize_bir(bir_bytes):
    """The walrus build here encodes at most ONE sync-wait per instruction
    ("Too many sync wait commands").  Tile attaches up to 3.  Split the
    extras onto EventSemaphore wait-carrier instructions inserted just
    before, on the same engine (engine streams keep BB relative order, so
    the carriers execute immediately before the original)."""
    import orjson

    d = orjson.loads(bir_bytes)
    for fn in d["functions"]:
        for bb in fn["blocks"]:
            out = []
            for inst in bb["instructions"]:
                si = inst.get("sync_info")
                ow = si.get("on_wait", []) if si else []
                if len(ow) > 1:
                    for j, w in enumerate(ow[:-1]):
                        out.append({
                            "debug": inst.get("debug", 0),
                            "engine": inst["engine"],
                            "ins": [],
                            "outs": [],
                            "name": f"{inst['name']}_wsplit{j}",
                            "opcode": "EventSemaphore",
                            "sync_info": {"on_update": [], "on_wait": [w]},
                        })
                    si["on_wait"] = [ow[-1]]
                out.append(inst)
            bb["instructions"] = out
    return orjson.dumps(d)


def _install_patches():
    import concourse.bass2jax as b2j
    from concourse.bass_utils import compile_bir_kernel as _cbk

    def _cbk_legal(bir_str, compile_dir_path, neff_name):
        return _cbk(_legalize_bir(bir_str), compile_dir_path,
                    neff_name=neff_name)

    b2j.compile_bir_kernel = _cbk_legal


def _build_program():
    import concourse.bass as bass
    import concourse.mybir as mybir
    import concourse.tile as tile
    import bass_rust
    from concourse.vector_clock import ScopedClock

    _install_patches()

    # The nix walrus build rejects >1 sync-wait on CTRL-class (Drain)
    # instructions; split the Tile tail-drain waits across a chain of
    # single-wait drains.
    def _drain_and_barrier(self, tick_clock, wait_clock):
        drain_inst = self.nc.sync.drain()
        wait_clock.add_sem_waits(
            drain_inst.ins, ScopedClock({None: tick_clock.global_clock})
        )
        si = drain_inst.ins.sync_info
        waits = list(si.on_wait) if si is not None else []
        if len(waits) > 1:
            si.on_wait = waits[:1]
            for w in waits[1:]:
                d = self.nc.sync.drain()
                d.ins.sync_info = bass_rust.SyncInfo(on_wait=[w], on_update=[])
        self.nc.all_engine_barrier()
        assert self.sems is not None
        popped = self.nc._tile_sem_poison_stack.pop()
        assert popped is self._sem_poison
        self.nc.clear_and_free_semaphores(list(self.sems.allocated().values()))
        self.nc.all_engine_barrier()

    tile.TileContext._drain_and_barrier = _drain_and_barrier

    f32 = mybir.dt.float32
    f16 = mybir.dt.float16
    Alu = mybir.AluOpType
    ms = _momentum_coeffs()

    nc = bass.Bass("TRN2", target_bir_lowering=False, debug=False,
                   num_devices=NCORES)
    d_sig = nc.dram_tensor("sig", [128, CW], f32, kind="ExternalInput").ap()
    d_w1 = nc.dram_tensor("w1", [128, 2 * K * 128], f16, kind="ExternalInput").ap()
    d_w2 = nc.dram_tensor("w2", [128, 2 * K * 128], f16, kind="ExternalInput").ap()
    d_wid = nc.dram_tensor("wid", [128, 2 * T * 128], f16, kind="ExternalInput").ap()
    d_out = nc.dram_tensor("xout", [128, K * CW], f32, kind="ExternalOutput").ap()

    with tile.TileContext(nc) as tc:
        with (
            tc.tile_pool(name="const", bufs=1) as const,
            tc.tile_pool(name="state", bufs=1) as state,
            tc.tile_pool(name="psq", bufs=2, space="PSUM") as psqp,
            tc.tile_pool(name="psu", bufs=2, space="PSUM") as psup,
        ):
            w1 = const.tile([128, 2 * K * 128], f16)
            w2 = const.tile([128, 2 * K * 128], f16)
            wid = const.tile([128, 2 * T * 128], f16)
            sigt = const.tile([128, CW], f32)
            nc.sync.dma_start(w1[:], d_w1[:])
            nc.sync.dma_start(w2[:], d_w2[:])
            nc.sync.dma_start(wid[:], d_wid[:])
            nc.sync.dma_start(sigt[:], d_sig[:])

            X32 = state.tile([128, K * CW], f32)
            XbA = state.tile([128, K * CWH], f16)
            XbB = state.tile([128, K * CWH], f16)
            rpA = state.tile([128, CW], f32)
            rpB = state.tile([128, CW], f32)
            v32 = state.tile([128, CWH], f32)
            v16 = state.tile([128, CWH], f16)
            btmp = state.tile([128, CW], f32)
            u_s = state.tile([128, K * CW], f32)
            tcl = state.tile([128, K * CW], f32)

            nc.gpsimd.memset(X32[:], 0.0)
            nc.gpsimd.memset(XbA[:], 0.0)
            nc.gpsimd.memset(XbB[:], 0.0)
            nc.vector.memset(rpB[:], 0.0)

            X32_3 = X32.rearrange("p (k c) -> p k c", k=K)

            def body(thr_f: float):
                for t in range(T):
                    m = float(ms[t])
                    Xc, Xp = (XbA, XbB) if t % 2 == 0 else (XbB, XbA)
                    rc, rp = (rpA, rpB) if t % 2 == 0 else (rpB, rpA)
                    Xc3 = Xc.rearrange("p (k c) -> p k c", k=K)
                    Xp3 = Xp.rearrange("p (k c) -> p k c", k=K)

                    # conv1: q = H @ x_t   (accumulate over k, 2 bands)
                    psq = psqp.tile([128, CW], f32)
                    for k in range(K):
                        nc.tensor.matmul(
                            psq[:],
                            w1[:, (2 * k) * 128:(2 * k + 1) * 128],
                            Xc[:, k * CWH + BL: (k + 1) * CWH],
                            start=(k == 0), stop=False,
                        )
                        nc.tensor.matmul(
                            psq[:],
                            w1[:, (2 * k + 1) * 128:(2 * k + 2) * 128],
                            Xc[:, k * CWH: k * CWH + CW],
                            start=False, stop=(k == K - 1),
                        )

                    # v = sig - (1+m) q + m r_prev   (w-momentum folded in)
                    nc.vector.scalar_tensor_tensor(
                        btmp[:], rp[:], m, sigt[:], Alu.mult, Alu.add)
                    nc.vector.scalar_tensor_tensor(
                        v32[:, 0:CW], psq[:], -(1.0 + m), btmp[:],
                        Alu.mult, Alu.add)
                    nc.scalar.copy(rc[:], psq[:])          # keep r_t for t+1
                    nc.gpsimd.tensor_copy(v32[:, CW:CWH], v32[:, 0:BL])  # halo
                    nc.scalar.copy(v16[:], v32[:])         # cast -> fp16

                    # conv2 + momentum passthrough, per-k PSUM regions.
                    # PSUM accumulation groups (start/stop + has_written
                    # clearing) are per BANK (512 f32 cols = 4 regions):
                    # start on the first matmul touching each bank, stop on
                    # the last.
                    psu = psup.tile([128, K * CW], f32)
                    for k in range(K):
                        reg = psu[:, k * CW:(k + 1) * CW]
                        nc.tensor.matmul(
                            reg, w2[:, (2 * k) * 128:(2 * k + 1) * 128],
                            v16[:, 0:CW], start=(k % 4 == 0), stop=False)
                        nc.tensor.matmul(
                            reg, w2[:, (2 * k + 1) * 128:(2 * k + 2) * 128],
                            v16[:, BL:CWH], start=False, stop=False)
                    for k in range(K):
                        nc.tensor.matmul(
                            psu[:, k * CW:(k + 1) * CW],
                            wid[:, (2 * t) * 128:(2 * t + 1) * 128],
                            Xc[:, k * CWH + BL:(k + 1) * CWH],
                            start=False, stop=False)
                    for k in range(K):
                        nc.tensor.matmul(
                            psu[:, k * CW:(k + 1) * CW],
                            wid[:, (2 * t + 1) * 128:(2 * t + 2) * 128],
                            Xp[:, k * CWH + BL:(k + 1) * CWH],
                            start=False, stop=(k % 4 == 3))

                    # softshrink: x_{t+1} = u - clamp(u, -thr, thr),
                    # u = x_t + psu
                    nc.vector.scalar_tensor_tensor(
                        u_s[:], psu[:], 1.0, X32[:], Alu.mult, Alu.add)
                    nc.vector.tensor_scalar(
                        tcl[:], u_s[:], -thr_f, thr_f, Alu.max, Alu.min)
                    nc.vector.tensor_sub(X32[:], u_s[:], tcl[:])
                    # fp16 copy (becomes next iteration's Xc) + halo
                    nc.scalar.copy(Xp3[:, :, BL:CWH], X32_3[:])
                    nc.gpsimd.tensor_copy(
                        Xp3[:, :, 0:BL], Xp3[:, :, CW:CWH])

            body(_CACHE["thr"])
            nc.sync.dma_start(d_out[:], X32[:])

    return nc


def kernel(signal, local_dictionary):
    sig = np.ascontiguousarray(np.asarray(signal, dtype=np.float32))
    D = np.ascontiguousarray(np.asarray(local_dictionary, dtype=np.float32))
    assert sig.shape == (N, B) and D.shape == (K, KS)

    # Lipschitz constant: H H^T = F^H diag(sum_k |fft(f_k)|^2) F  (circulants)
    fpad = np.zeros((K, N), np.float64)
    fpad[:, :KS] = D.astype(np.float64)
    L = np.float32((np.abs(np.fft.fft(fpad, axis=1)) ** 2).sum(0).max() + 1.0)
    thr = np.float32(LAM / L)
    _CACHE["thr"] = float(thr)

    Dm, Sm = _band_matrices(D)
    ms = _momentum_coeffs()

    # conv1 lhsT[j,i] = D_k[i,j]  (transposed);  conv2 lhsT[i,j] = D_k[i,j]/L
    w1 = np.empty((128, 2 * K * 128), np.float16)
    w2 = np.empty((128, 2 * K * 128), np.float16)
    for k in range(K):
        w1[:, (2 * k) * 128:(2 * k + 1) * 128] = Dm[k].T.astype(np.float16)
        w1[:, (2 * k + 1) * 128:(2 * k + 2) * 128] = Sm[k].T.astype(np.float16)
        w2[:, (2 * k) * 128:(2 * k + 1) * 128] = (Dm[k] / L).astype(np.float16)
        w2[:, (2 * k + 1) * 128:(2 * k + 2) * 128] = (Sm[k] / L).astype(np.float16)
    eye = np.eye(128, dtype=np.float32)
    wid = np.empty((128, 2 * T * 128), np.float16)
    for t in range(T):
        wid[:, (2 * t) * 128:(2 * t + 1) * 128] = (ms[t] * eye).astype(np.float16)
        wid[:, (2 * t + 1) * 128:(2 * t + 2) * 128] = (-ms[t] * eye).astype(np.float16)

    nc = _build_program()

    from concourse.bass_utils import run_bass_kernel_spmd

    in_maps = []
    for c in range(NCORES):
        sc = sig[:, c * BL:(c + 1) * BL]                      # [2048, 8]
        sc = sc.reshape(NB, 128, BL).transpose(1, 0, 2).reshape(128, CW)
        in_maps.append({
            "sig": np.ascontiguousarray(sc),
            "w1": w1, "w2": w2, "wid": wid,
        })

    _CACHE["in_maps"] = in_maps
    res = run_bass_kernel_spmd(nc, in_maps, list(range(NCORES)))

    out = np.empty((K * N, B), np.float32)
    for c in range(NCORES):
        xc = res.results[c]["xout"]                           # [128, 1024]
        xc = xc.reshape(128, K, NB, BL).transpose(1, 2, 0, 3).reshape(K * N, BL)
        out[:, c * BL:(c + 1) * BL] = xc
    return out
